# revision 35
# baseline (speedup 1.0000x reference)
"""Trainium2 Bass kernel for EntityAttention.

    beta[b,e,a] = (agent[b,e] @ w_psi) . (vis[b,e,a] @ w_phi)
    out         = softmax_a(beta)

v3: the per-row dot products run on the TENSOR engine as an fp16
cross-product, not on DVE scans.

    qT[k, be]    = sum_d w_psi[d, k] agT[d, be]                  (PE, f32)
    tT[d2, be]   = sum_k w_phiT[k, d2] qT[k, be]                 (PE, f32->fp16)
    bx[be',(be,a)] = sum_d tT16[d, be'] visT16[d, (be, a)]       (PE, fp16, PSUM)
    beta[be, a]  = bx[be, (be, a)]                               (diagonal)

The host pre-transposes: agT = agent.T (f32) and visT16 = fp16(vis)
laid out [d, (be, a)] so the PE streams it directly as moving data with
tT16 chunks stationary — the huge vis tensor is read exactly once, in
half precision, and the batched per-row dot product becomes a dense
matmul whose diagonal blocks are the betas.

Diagonal extraction: DVE runs ONE fused multiply+cumsum per be-chunk
over the PSUM cross tile with an a-major access pattern, masked by the
identity matrix (in1 = I[p, be] broadcast over a). The running sum then
increments by beta[p, a] exactly once per 128-element page, so the page
boundary values (fixed free positions!) are prefix sums of the betas and
one subtract recovers them. This replaces ~35us of DVE scans with ~9us.

fp16 rounding (vis and tT) perturbs each beta by sigma ~ 2; the softmax
is near-one-hot so only near-ties matter. Per row the kernel extracts
the top-2 (value, index) with tiny custom DVE reduce ops, gathers the
two exact f32 vis rows by indirect DMA, recomputes those betas exactly
against f32 t, and patches them — restoring the reference ranking.

Sharding: data-parallel over the batch axis across 8 NeuronCores
(16 batches / core); w_psi / w_phi replicated.
"""

from contextlib import ExitStack

import numpy as np

import concourse.bass as bass
import concourse.tile as tile
from concourse import bacc, bass_utils, dve_ops, mybir
from concourse.dve_spec import (
    AluOp, Spec, Src0, Src1, C0, C1, C2, C3, Zero, One, MaxNeg,
    eq, ne, select, Idx, _has_src1, lower, scan, _spill_c3_to_src1,
)
from concourse.dve_uop import DveOpSpec
from concourse.masks import make_identity

# Problem shape (hardcoded per contract; kernel.py must be self-contained).
B, E, A, D, K = 128, 32, 16, 512, 128
N_CORES = 8
B_SH = B // N_CORES          # batches per core = 16
BE = B_SH * E                # rows per core = 512
NBC = BE // 128              # be-chunks of 128 partitions = 4
NDC = D // 128               # d-chunks = 4
NG = 4                       # 512-col groups per cross tile (2048/512)
BIG = 1.0e9
F32 = mybir.dt.float32
F16 = mybir.dt.float16
I32 = mybir.dt.int32


# ---- custom DVE ops ------------------------------------------------------ #

def _ref_cumsum_mul(in0, in1, s0, s1, imm2):
    p = in0.shape[0]
    a = np.asarray(in0, np.float32).reshape(p, -1)
    b = np.ascontiguousarray(np.asarray(in1, np.float32)).reshape(p, -1)
    if b.shape[1] != a.shape[1]:
        b = np.tile(b, (1, a.shape[1] // b.shape[1]))
    init = s0 if isinstance(s0, np.ndarray) else np.float32(s0)
    return init + np.cumsum(a * b, axis=-1, dtype=np.float32)


def _ref_bmax(in0, in1, s0, s1, imm2):
    p = in0.shape[0]
    x = np.asarray(in0, np.float32).reshape(p, -1)
    return x, x.max(axis=-1, keepdims=True)


def _c3(in1, p):
    # C3 is spilled through in1, latched at element 0.
    return np.asarray(in1, np.float32).reshape(p, -1)[:, 0:1]


def _ref_idx0rb_s1(in0, in1, s0, s1, imm2):
    p = in0.shape[0]
    x = np.asarray(in0, np.float32).reshape(p, -1)
    rb = np.asarray(s1, np.float32).reshape(p, 1)
    idx = np.broadcast_to(np.arange(x.shape[1], dtype=np.float32), x.shape) + rb
    m = np.asarray(s0, np.float32).reshape(p, 1)
    out = np.where(x == m, idx, np.float32(imm2))
    return out, np.minimum(out.min(axis=-1, keepdims=True), np.float32(imm2))


def _ref_m2rb(in0, in1, s0, s1, imm2):
    p = in0.shape[0]
    x = np.asarray(in0, np.float32).reshape(p, -1)
    rb = np.asarray(s1, np.float32).reshape(p, 1)
    idx = np.broadcast_to(np.arange(x.shape[1], dtype=np.float32), x.shape) + rb
    i0 = np.asarray(s0, np.float32).reshape(p, 1)
    out = np.where(idx == i0, -np.finfo(np.float32).max, x)
    return out, out.max(axis=-1, keepdims=True)


def _ref_idx1(in0, in1, s0, s1, imm2):
    p = in0.shape[0]
    x = np.asarray(in0, np.float32).reshape(p, -1)
    idx = np.broadcast_to(np.arange(x.shape[1], dtype=np.float32), x.shape)
    m = np.asarray(s0, np.float32).reshape(p, 1)
    i0 = np.asarray(s1, np.float32).reshape(p, 1)
    out = np.where((x == m) & (idx != i0), idx, np.float32(imm2))
    return out, np.minimum(out.min(axis=-1, keepdims=True), np.float32(imm2))


def _ref_patch_set_rb(in0, in1, s0, s1, imm2):
    p = in0.shape[0]
    x = np.asarray(in0, np.float32).reshape(p, -1).copy()
    rb = _c3(in1, p)
    idx = np.broadcast_to(np.arange(x.shape[1], dtype=np.float32), x.shape) + rb
    i0 = np.asarray(s0, np.float32).reshape(p, 1)
    v = np.asarray(s1, np.float32).reshape(p, 1)
    return np.where(idx == i0, v, x)


def _register(name, spec):
    if name in dve_ops._SUB_OPCODE_FOR_NAME:
        return next(op for op in dve_ops.OPS if op.name == name)
    row = dve_ops._CUSTOM_DVE_ROW_BASE + len(dve_ops.OPS)
    assert row < 0x20
    shas = {}
    for ver in ("v3", "v4"):
        d = DveOpSpec(name=name, opcode=row, uops=lower(spec, ver=ver),
                      rd1_en=_has_src1(spec))
        shas[ver] = d.sha(ver)
    op = dve_ops.DveOp(name, spec, subdim=False, uops_sha=shas)
    dve_ops._SUB_OPCODE_FOR_NAME[name] = row
    dve_ops.OPS.append(op)
    dve_ops.CUSTOM_DVE_SPECS[name] = spec
    return op


# out = cumsum(in0 * in1) along the free axis (f32 accumulation)
CUMSUM_MUL = _register(
    "CUMSUM_MUL_ANT",
    Spec(body=scan(AluOp.ADD, Src0 * Src1, init=C0), reference=_ref_cumsum_mul))
# accum_out = max(in0)
BMAX = _register(
    "BMAX_ANT",
    Spec(body=Src0 * One, accum=AluOp.MAX, reference=_ref_bmax))
# accum_out = min over {(index + rb) where in0 == s0} (rb = s1, imm2 =
# sentinel > any global index) — the GLOBAL row index of the max.
IDX0RB = _register(
    "IDX0RB_ANT",
    Spec(body=select(eq(Src0, C0), Idx + C1, C2),
         accum=AluOp.MIN, accum_init=C2, reference=_ref_idx0rb_s1))
# accum_out = max(in0 with global position s0 masked out); s1 = rb
M2RB = _register(
    "M2RB_ANT",
    Spec(body=select(eq(Idx + C1, C0), MaxNeg, Src0), accum=AluOp.MAX,
         reference=_ref_m2rb))
# accum_out = first LOCAL index where in0 == s0 and global index != s1
# (s1 = i0 global; local idx + rb == s1 <=> select excluded via M2's mask,
# so comparing against the masked max value m1 with local-index output and
# a trailing add keeps the spill single-site)
IDX1 = _register(
    "IDX1_ANT",
    Spec(body=select(eq(Src0, C0) & ne(Idx, C1), Idx, C2), accum=AluOp.MIN,
         accum_init=C2, reference=_ref_idx1))
# out = (global index == s0 ? s1 : in0)  (replace one element per partition)
PATCH_SET_RB = _register(
    "PATCH_SET_RB_ANT",
    Spec(body=_spill_c3_to_src1(select(eq(Idx + C3, C0), C1, Src0)),
         reference=_ref_patch_set_rb))


def _bcast_mid(ap_2d, count):
    """[P, N] AP -> [P, count, N] AP with a step-0 middle dim."""
    return bass.AP(
        tensor=ap_2d.tensor,
        offset=ap_2d.offset,
        ap=[ap_2d.ap[0], [0, count], *ap_2d.ap[1:]],
    )


def _emit(tc, nc, agT_d, vt_d, vf_d, wpsi_d, wphi_d, rb_d, out_d):
    with ExitStack() as ctx:
        const = ctx.enter_context(tc.tile_pool(name="const", bufs=1))
        vtp = ctx.enter_context(tc.tile_pool(name="vtp", bufs=16))
        cump = ctx.enter_context(tc.tile_pool(name="cump", bufs=2))
        gp = ctx.enter_context(tc.tile_pool(name="gp", bufs=2))
        rcp = ctx.enter_context(tc.tile_pool(name="rcp", bufs=2))
        small = ctx.enter_context(tc.tile_pool(name="small", bufs=4))

        ident = const.tile([128, 128], F32)
        make_identity(nc, ident)

        # qt-chain inputs lead the sync (SP) queue in consumption order:
        # wpsi, then the four agT chunks (plain slices), then wphi; rowbase
        # rides the scalar queue. Everything is f32 here — q/t must be
        # exact, only the stationary tT and the vis stream go fp16.
        # wpsi arrives pre-chunked from the host as [p, r, k] = wpsi[r*128+p, k]
        # so every partition line is one contiguous 2KB descriptor.
        wpsi_sb = const.tile([128, NDC, K], F32)
        nc.sync.dma_start(out=wpsi_sb, in_=wpsi_d)
        wphi_sb = const.tile([128, NDC, K], F32)
        nc.sync.dma_start(out=wphi_sb, in_=wphi_d)
        agT_tiles = []
        for r in range(NDC):
            agt = const.tile([128, BE], F32, tag=f"agT{r}", name=f"agT{r}")
            nc.sync.dma_start(out=agt, in_=agT_d[r * 128:(r + 1) * 128, :])
            agT_tiles.append(agt)
        rb_sb = const.tile([128, NBC], F32)
        nc.scalar.dma_start(out=rb_sb, in_=rb_d)

        # visT16 stream on the sync queue, in consumption order (c outer,
        # d-chunk inner). bufs=16 keeps every transfer enqueued from t=0.
        vt_tiles = {}
        for c in range(NBC):
            for r in range(NDC):
                vt = vtp.tile([128, 2048], F16, tag="vt", name=f"vt{c}_{r}")
                nc.sync.dma_start(
                    out=vt, in_=vt_d[r * 128:(r + 1) * 128, c * 2048:(c + 1) * 2048])
                vt_tiles[(c, r)] = vt

        # Prologue PSUM lives in its own pools, closed before the main
        # cross-product pool opens (which needs all 8 banks).
        t_tiles = []
        tT_tiles = []
        with tc.tile_pool(name="pro_tr", bufs=2, space="PSUM") as pro_tr, \
             tc.tile_pool(name="pro_mm", bufs=2, space="PSUM") as pro_mm:
            # Warm the PE clock (HAM) so the t chain runs at 2.4 GHz; the
            # warmups fill the window until the weight DMAs land.
            for wup in range(6):
                warm = pro_tr.tile([128, 128], F32, tag="tr", name=f"warm{wup}")
                nc.tensor.transpose(warm, ident, ident)

            # w_phiT with natural dout order: wphiT[k, dl, r] = w_phi[4*dl+r, k],
            # flat free index f = dl*4 + r = dout.
            wphiT_sb = const.tile([128, 128, NDC], F32)
            for r in range(NDC):
                tr = pro_tr.tile([128, 128], F32, tag="tr", name=f"trw{r}")
                nc.tensor.transpose(tr, wphi_sb[:, r, :], ident)
                nc.scalar.copy(wphiT_sb[:, :, r], tr)

            # qT[k, be] = sum_r w_psi_chunk_r.T @ agT_chunk_r  (plain d-chunks)
            qt_ps = pro_mm.tile([128, BE], F32, tag="qt", name="qt")
            for r in range(NDC):
                nc.tensor.matmul(
                    qt_ps, lhsT=wpsi_sb[:, r, :], rhs=agT_tiles[r],
                    start=(r == 0), stop=(r == NDC - 1))
            qT_sb = const.tile([128, BE], F32)
            nc.scalar.copy(qT_sb, qt_ps)

            # tT16[r][dp, be] = fp16(t[be, r*128+dp]) — the fp16 stationary.
            for r in range(NDC):
                tt_ps = pro_mm.tile([128, BE], F32, tag="tt", name=f"tt{r}")
                wslice = wphiT_sb[:, r * 32:(r + 1) * 32, :].rearrange(
                    "p a b -> p (a b)")
                nc.tensor.matmul(tt_ps, lhsT=wslice, rhs=qT_sb, start=True, stop=True)
                tt16 = const.tile([128, BE], F16, tag=f"tt16_{r}", name=f"tt16_{r}")
                if r % 2 == 0:
                    nc.scalar.copy(tt16, tt_ps)
                else:
                    nc.vector.tensor_copy(tt16, tt_ps)
                tT_tiles.append(tt16)

        # Main PSUM: cross half-tiles (2 banks each, 3 in flight) plus a
        # 1-bank slot for the per-chunk t matmul (needed only by refine,
        # computed inside the main loop to stay off the prologue chain).
        bx_pool = ctx.enter_context(tc.tile_pool(name="bx", bufs=3, space="PSUM"))
        t_pool = ctx.enter_context(tc.tile_pool(name="tp", bufs=1, space="PSUM"))

        beta = {}
        topk = {}
        exps = {}

        def emit_mm(c):
            """Cross half-tiles bxh[be', (be, a-half)] = sum_d tT16 visT16,
            one per a-half so each PSUM tile is 2 banks. The per-chunk t
            (f32, for refine) rides along after the halves."""
            bxs = []
            for h in range(2):
                bxh = bx_pool.tile([128, 1024], F32, tag="bx", name=f"bx{c}_{h}")
                for r in range(NDC):
                    vt_v = vt_tiles[(c, r)].rearrange("p (b a) -> p b a", a=A)
                    for s in range(2):
                        nc.tensor.matmul(
                            bxh[:, s * 512:(s + 1) * 512],
                            lhsT=tT_tiles[r][:, c * 128:(c + 1) * 128],
                            rhs=vt_v[:, s * 64:(s + 1) * 64, h * 8:(h + 1) * 8],
                            start=(r == 0), stop=(r == NDC - 1))
                bxs.append(bxh)
            t_ps = t_pool.tile([128, D], F32, tag="t", name=f"t{c}")
            nc.tensor.matmul(
                t_ps, lhsT=qT_sb[:, c * 128:(c + 1) * 128],
                rhs=wphiT_sb[:, :, :], start=True, stop=True)
            t_sb = const.tile([128, D], F32, tag=f"t{c}", name=f"tsb{c}")
            nc.scalar.copy(t_sb, t_ps)
            t_tiles.append(t_sb)
            return bxs

        def emit_beta(c, bxs):
            """Masked cumsum diagonal extraction. a-major AP over each PSUM
            half-tile x identity mask: the running sum increments by
            beta[p, a] once per 128-element page, so page-boundary values
            (fixed positions) are prefix sums of the betas."""
            beta_sb = small.tile([128, A], F32, tag="beta", name=f"beta{c}")
            for h in range(2):
                cum = cump.tile([128, 8, 128], F32, tag="cum", name=f"cum{c}_{h}")
                nc.vector._custom_dve(
                    CUMSUM_MUL, out=cum,
                    in0=bxs[h].rearrange("p (b a) -> p a b", a=8),
                    in1=_bcast_mid(ident, 8), s0=0.0)
                bnd = cum[:, :, 127:128].rearrange("p s o -> p (s o)")
                o = h * 8
                nc.vector.tensor_copy(beta_sb[:, o:o + 1], bnd[:, 0:1])
                nc.vector.tensor_sub(
                    beta_sb[:, o + 1:o + 8], bnd[:, 1:8], bnd[:, 0:7])
            beta[c] = beta_sb

        def emit_topk_gather(c):
            """DVE top-2 (value, index) extraction; DVE casts the indices and
            GPSIMD runs the two indirect f32 row gathers. The first gather is
            issued as soon as i0 is known, before the second max pass."""
            scr = small.tile([128, A], F32, tag="scr", name=f"scr{c}")
            m0 = small.tile([128, 1], F32, tag="m0", name=f"m0{c}")
            m1 = small.tile([128, 1], F32, tag="m1", name=f"m1{c}")
            idxg = small.tile([128, 2], F32, tag="idxg", name=f"idxg{c}")
            idxi = small.tile([128, 2], I32, tag="idxi", name=f"idxi{c}")
            rb = rb_sb[:, c:c + 1]
            g = gp.tile([128, 2, D], F32, tag="g", name=f"g{c}")
            nc.vector._custom_dve(BMAX, out=scr, in0=beta[c], accum_out=m0)
            nc.vector._custom_dve(IDX0RB, out=scr, in0=beta[c], s0=m0, s1=rb,
                                  imm2=BIG, accum_out=idxg[:, 0:1])
            nc.scalar.copy(idxi[:, 0:1], idxg[:, 0:1])
            nc.gpsimd.indirect_dma_start(
                out=g[:, 0, :], out_offset=None, in_=vf_d,
                in_offset=bass.IndirectOffsetOnAxis(ap=idxi[:, 0:1], axis=0),
            )
            nc.vector._custom_dve(M2RB, out=scr, in0=beta[c], s0=idxg[:, 0:1],
                                  s1=rb, accum_out=m1)
            i0l = small.tile([128, 1], F32, tag="i0l", name=f"i0l{c}")
            nc.vector.tensor_sub(i0l, idxg[:, 0:1], rb)
            i1l = small.tile([128, 1], F32, tag="i1l", name=f"i1l{c}")
            nc.vector._custom_dve(IDX1, out=scr, in0=beta[c], s0=m1,
                                  s1=i0l, imm2=BIG, accum_out=i1l)
            nc.vector.tensor_add(idxg[:, 1:2], i1l, rb)
            nc.scalar.copy(idxi[:, 1:2], idxg[:, 1:2])
            nc.gpsimd.indirect_dma_start(
                out=g[:, 1, :], out_offset=None, in_=vf_d,
                in_offset=bass.IndirectOffsetOnAxis(ap=idxi[:, 1:2], axis=0),
            )
            topk[c] = (idxg, g, m0)

        def emit_refine(c):
            """Recompute the two top betas exactly from the gathered f32 rows
            (one fused scan over both) and overwrite them in place."""
            idxg, g, _ = topk[c]
            rb = rb_sb[:, c:c + 1]
            rc = rcp.tile([128, 2, D], F32, tag="rc", name=f"rc{c}")
            nc.vector._custom_dve(
                CUMSUM_MUL, out=rc, in0=g, in1=_bcast_mid(t_tiles[c], 2))
            b1 = small.tile([128, 1], F32, tag="b1", name=f"b1{c}")
            nc.vector.tensor_sub(b1, rc[:, 1, D - 1:D], rc[:, 0, D - 1:D])
            nc.vector._custom_dve(PATCH_SET_RB, out=beta[c], in0=beta[c],
                                  in1=rb, s0=idxg[:, 0:1],
                                  s1=rc[:, 0, D - 1:D])
            nc.vector._custom_dve(PATCH_SET_RB, out=beta[c], in0=beta[c],
                                  in1=rb, s0=idxg[:, 1:2], s1=b1)

        def emit_exp(c):
            # exp bias needs only ~max(beta); the pre-refine max m0 is within
            # ~sigma of it. Negate on the Scalar engine.
            negm = small.tile([128, 1], F32, tag="negm", name=f"negm{c}")
            nc.scalar.mul(negm, topk[c][2], -1.0)
            prob = small.tile([128, A], F32, tag="prob", name=f"prob{c}")
            ssum = small.tile([128, 1], F32, tag="ssum", name=f"ssum{c}")
            nc.scalar.activation(
                prob, beta[c], mybir.ActivationFunctionType.Exp,
                bias=negm, scale=1.0, accum_out=ssum,
            )
            exps[c] = (prob, ssum)

        def emit_finish(c):
            cs = slice(c * 128, (c + 1) * 128)
            prob, ssum = exps[c]
            rec = small.tile([128, 1], F32, tag="rec", name=f"rec{c}")
            nc.vector.reciprocal(rec, ssum)
            osb = small.tile([128, A], F32, tag="osb", name=f"osb{c}")
            nc.scalar.mul(osb, prob, rec)
            nc.scalar.dma_start(out=out_d[cs, :], in_=osb)

        # Software pipeline: chunk c's refine (which waits on its gather
        # round-trip) runs TWO chunks later, in its own tick after chunk
        # c+2's extraction, so the in-order DVE never stalls on a gather;
        # finishes trail one more chunk.
        for c in range(NBC):
            with tc.tile_wait_until(2 * c):
                bx = emit_mm(c)
                emit_beta(c, bx)
                emit_topk_gather(c)
            with tc.tile_wait_until(2 * c + 1):
                if c >= 2:
                    emit_refine(c - 2)
                    emit_exp(c - 2)
                if c >= 3:
                    emit_finish(c - 3)
        with tc.tile_wait_until(2 * NBC):
            emit_refine(NBC - 2)
            emit_exp(NBC - 2)
            emit_finish(NBC - 3)
        with tc.tile_wait_until(2 * NBC + 1):
            emit_refine(NBC - 1)
            emit_exp(NBC - 1)
            emit_finish(NBC - 2)
            emit_finish(NBC - 1)


def _build_program():
    nc = bacc.Bacc("TRN2", target_bir_lowering=False, debug=False)
    agT_d = nc.dram_tensor("agT", (D, BE), F32, kind="ExternalInput").ap()
    vt_d = nc.dram_tensor("vt", (D, BE * A), F16, kind="ExternalInput").ap()
    vf_d = nc.dram_tensor("vis_f", (BE * A, D), F32, kind="ExternalInput").ap()
    wpsi_d = nc.dram_tensor("w_psi", (128, NDC * K), F32, kind="ExternalInput").ap()
    wphi_d = nc.dram_tensor("w_phi", (128, NDC * K), F32, kind="ExternalInput").ap()
    rb_d = nc.dram_tensor("rowbase", (128, NBC), F32, kind="ExternalInput").ap()
    out_d = nc.dram_tensor("out", (BE, A), F32, kind="ExternalOutput").ap()
    with tile.TileContext(nc) as tc:
        _emit(tc, nc, agT_d, vt_d, vf_d, wpsi_d, wphi_d, rb_d, out_d)
    nc.compile()
    return nc


_PROG = None


def _get_program():
    global _PROG
    if _PROG is None:
        _PROG = _build_program()
    return _PROG


_ROWBASE = (
    (np.arange(NBC, dtype=np.float32)[None, :] * 128
     + np.arange(128, dtype=np.float32)[:, None]) * A
).astype(np.float32)


def make_in_maps(agent_observation, visible_observations, w_psi, w_phi):
    agent = np.ascontiguousarray(np.asarray(agent_observation, np.float32)).reshape(B, E, D)
    vis = np.ascontiguousarray(np.asarray(visible_observations, np.float32)).reshape(B, E, A, D)
    wpsi = np.asarray(w_psi, np.float32)
    # pre-chunked layout: wpsi_x[p, r*K+k] = wpsi[r*128+p, k] (2KB DMA lines)
    wpsi_x = np.ascontiguousarray(
        wpsi.reshape(NDC, 128, K).transpose(1, 0, 2).reshape(128, NDC * K))
    wphi = np.asarray(w_phi, np.float32)
    # interleaved layout: wphi_x[p, r*K+k] = wphi[4p+r, k] (2KB DMA lines)
    wphi_x = np.ascontiguousarray(wphi.reshape(128, NDC * K))
    in_maps = []
    for ci in range(N_CORES):
        sl = slice(ci * B_SH, (ci + 1) * B_SH)
        v = vis[sl].reshape(BE, A, D)
        v16 = v.astype(np.float16)
        vt = np.ascontiguousarray(v16.transpose(2, 0, 1).reshape(D, BE * A))
        agT = np.ascontiguousarray(agent[sl].reshape(BE, D).T)
        in_maps.append({
            "agT": agT,
            "vt": vt,
            "vis_f": np.ascontiguousarray(v.reshape(BE * A, D)),
            "w_psi": wpsi_x,
            "w_phi": wphi_x,
            "rowbase": _ROWBASE,
        })
    return in_maps


def run_sharded(in_maps, trace=False, **kwargs):
    nc = _get_program()
    return bass_utils.run_bass_kernel_spmd(
        nc, in_maps, core_ids=list(range(N_CORES)), trace=trace, **kwargs
    )


def kernel(agent_observation, visible_observations, w_psi, w_phi):
    in_maps = make_in_maps(agent_observation, visible_observations, w_psi, w_phi)
    res = run_sharded(in_maps)
    return np.concatenate(
        [r["out"].reshape(B_SH, E, A) for r in res.results], axis=0
    )


# revision 38
# speedup vs baseline: 1.0541x; 1.0541x over previous
"""Trainium2 Bass kernel for EntityAttention.

    beta[b,e,a] = (agent[b,e] @ w_psi) . (vis[b,e,a] @ w_phi)
    out         = softmax_a(beta)

v3: the per-row dot products run on the TENSOR engine as an fp16
cross-product, not on DVE scans.

    qT[k, be]    = sum_d w_psi[d, k] agT[d, be]                  (PE, f32)
    tT[d2, be]   = sum_k w_phiT[k, d2] qT[k, be]                 (PE, f32->fp16)
    bx[be',(be,a)] = sum_d tT16[d, be'] visT16[d, (be, a)]       (PE, fp16, PSUM)
    beta[be, a]  = bx[be, (be, a)]                               (diagonal)

The host pre-transposes: agT = agent.T (f32) and visT16 = fp16(vis)
laid out [d, (be, a)] so the PE streams it directly as moving data with
tT16 chunks stationary — the huge vis tensor is read exactly once, in
half precision, and the batched per-row dot product becomes a dense
matmul whose diagonal blocks are the betas.

Diagonal extraction: DVE runs ONE fused multiply+cumsum per be-chunk
over the PSUM cross tile with an a-major access pattern, masked by the
identity matrix (in1 = I[p, be] broadcast over a). The running sum then
increments by beta[p, a] exactly once per 128-element page, so the page
boundary values (fixed free positions!) are prefix sums of the betas and
one subtract recovers them. This replaces ~35us of DVE scans with ~9us.

fp16 rounding (vis and tT) perturbs each beta by sigma ~ 2; the softmax
is near-one-hot so only near-ties matter. Per row the kernel extracts
the top-2 (value, index) with tiny custom DVE reduce ops, gathers the
two exact f32 vis rows by indirect DMA, recomputes those betas exactly
against f32 t, and patches them — restoring the reference ranking.

Sharding: data-parallel over the batch axis across 8 NeuronCores
(16 batches / core); w_psi / w_phi replicated.
"""

from contextlib import ExitStack

import numpy as np

import concourse.bass as bass
import concourse.tile as tile
from concourse import bacc, bass_utils, dve_ops, mybir
from concourse.dve_spec import (
    AluOp, Spec, Src0, Src1, C0, C1, C2, C3, Zero, One, MaxNeg,
    eq, ne, select, Idx, _has_src1, lower, scan, _spill_c3_to_src1,
)
from concourse.dve_uop import DveOpSpec
from concourse.masks import make_identity

# Problem shape (hardcoded per contract; kernel.py must be self-contained).
B, E, A, D, K = 128, 32, 16, 512, 128
N_CORES = 8
B_SH = B // N_CORES          # batches per core = 16
BE = B_SH * E                # rows per core = 512
NBC = BE // 128              # be-chunks of 128 partitions = 4
NDC = D // 128               # d-chunks = 4
NG = 4                       # 512-col groups per cross tile (2048/512)
BIG = 1.0e9
F32 = mybir.dt.float32
F16 = mybir.dt.float16
I32 = mybir.dt.int32


# ---- custom DVE ops ------------------------------------------------------ #

def _ref_cumsum_mul(in0, in1, s0, s1, imm2):
    p = in0.shape[0]
    a = np.asarray(in0, np.float32).reshape(p, -1)
    b = np.ascontiguousarray(np.asarray(in1, np.float32)).reshape(p, -1)
    if b.shape[1] != a.shape[1]:
        b = np.tile(b, (1, a.shape[1] // b.shape[1]))
    init = s0 if isinstance(s0, np.ndarray) else np.float32(s0)
    return init + np.cumsum(a * b, axis=-1, dtype=np.float32)


def _ref_bmax(in0, in1, s0, s1, imm2):
    p = in0.shape[0]
    x = np.asarray(in0, np.float32).reshape(p, -1)
    return x, x.max(axis=-1, keepdims=True)


def _c3(in1, p):
    # C3 is spilled through in1, latched at element 0.
    return np.asarray(in1, np.float32).reshape(p, -1)[:, 0:1]


def _ref_idx0rb_s1(in0, in1, s0, s1, imm2):
    p = in0.shape[0]
    x = np.asarray(in0, np.float32).reshape(p, -1)
    rb = np.asarray(s1, np.float32).reshape(p, 1)
    idx = np.broadcast_to(np.arange(x.shape[1], dtype=np.float32), x.shape) + rb
    m = np.asarray(s0, np.float32).reshape(p, 1)
    out = np.where(x == m, idx, np.float32(imm2))
    return out, np.minimum(out.min(axis=-1, keepdims=True), np.float32(imm2))


def _ref_m2rb(in0, in1, s0, s1, imm2):
    p = in0.shape[0]
    x = np.asarray(in0, np.float32).reshape(p, -1)
    rb = np.asarray(s1, np.float32).reshape(p, 1)
    idx = np.broadcast_to(np.arange(x.shape[1], dtype=np.float32), x.shape) + rb
    i0 = np.asarray(s0, np.float32).reshape(p, 1)
    out = np.where(idx == i0, -np.finfo(np.float32).max, x)
    return out, out.max(axis=-1, keepdims=True)


def _ref_idx1(in0, in1, s0, s1, imm2):
    p = in0.shape[0]
    x = np.asarray(in0, np.float32).reshape(p, -1)
    idx = np.broadcast_to(np.arange(x.shape[1], dtype=np.float32), x.shape)
    m = np.asarray(s0, np.float32).reshape(p, 1)
    i0 = np.asarray(s1, np.float32).reshape(p, 1)
    out = np.where((x == m) & (idx != i0), idx, np.float32(imm2))
    return out, np.minimum(out.min(axis=-1, keepdims=True), np.float32(imm2))


def _ref_patch_set_rb(in0, in1, s0, s1, imm2):
    p = in0.shape[0]
    x = np.asarray(in0, np.float32).reshape(p, -1).copy()
    rb = _c3(in1, p)
    idx = np.broadcast_to(np.arange(x.shape[1], dtype=np.float32), x.shape) + rb
    i0 = np.asarray(s0, np.float32).reshape(p, 1)
    v = np.asarray(s1, np.float32).reshape(p, 1)
    return np.where(idx == i0, v, x)


def _register(name, spec):
    if name in dve_ops._SUB_OPCODE_FOR_NAME:
        return next(op for op in dve_ops.OPS if op.name == name)
    row = dve_ops._CUSTOM_DVE_ROW_BASE + len(dve_ops.OPS)
    assert row < 0x20
    shas = {}
    for ver in ("v3", "v4"):
        d = DveOpSpec(name=name, opcode=row, uops=lower(spec, ver=ver),
                      rd1_en=_has_src1(spec))
        shas[ver] = d.sha(ver)
    op = dve_ops.DveOp(name, spec, subdim=False, uops_sha=shas)
    dve_ops._SUB_OPCODE_FOR_NAME[name] = row
    dve_ops.OPS.append(op)
    dve_ops.CUSTOM_DVE_SPECS[name] = spec
    return op


# out = cumsum(in0 * in1) along the free axis (f32 accumulation)
CUMSUM_MUL = _register(
    "CUMSUM_MUL_ANT",
    Spec(body=scan(AluOp.ADD, Src0 * Src1, init=C0), reference=_ref_cumsum_mul))
# accum_out = max(in0)
BMAX = _register(
    "BMAX_ANT",
    Spec(body=Src0 * One, accum=AluOp.MAX, reference=_ref_bmax))
# accum_out = min over {(index + rb) where in0 == s0} (rb = s1, imm2 =
# sentinel > any global index) — the GLOBAL row index of the max.
IDX0RB = _register(
    "IDX0RB_ANT",
    Spec(body=select(eq(Src0, C0), Idx + C1, C2),
         accum=AluOp.MIN, accum_init=C2, reference=_ref_idx0rb_s1))
# accum_out = max(in0 with global position s0 masked out); s1 = rb
M2RB = _register(
    "M2RB_ANT",
    Spec(body=select(eq(Idx + C1, C0), MaxNeg, Src0), accum=AluOp.MAX,
         reference=_ref_m2rb))
# accum_out = first LOCAL index where in0 == s0 and global index != s1
# (s1 = i0 global; local idx + rb == s1 <=> select excluded via M2's mask,
# so comparing against the masked max value m1 with local-index output and
# a trailing add keeps the spill single-site)
IDX1 = _register(
    "IDX1_ANT",
    Spec(body=select(eq(Src0, C0) & ne(Idx, C1), Idx, C2), accum=AluOp.MIN,
         accum_init=C2, reference=_ref_idx1))
# out = (global index == s0 ? s1 : in0)  (replace one element per partition)
PATCH_SET_RB = _register(
    "PATCH_SET_RB_ANT",
    Spec(body=_spill_c3_to_src1(select(eq(Idx + C3, C0), C1, Src0)),
         reference=_ref_patch_set_rb))


def _bcast_mid(ap_2d, count):
    """[P, N] AP -> [P, count, N] AP with a step-0 middle dim."""
    return bass.AP(
        tensor=ap_2d.tensor,
        offset=ap_2d.offset,
        ap=[ap_2d.ap[0], [0, count], *ap_2d.ap[1:]],
    )


def _emit(tc, nc, agT_d, vt_d, vf_d, wpsi_d, wphi_d, rb_d, out_d):
    with ExitStack() as ctx:
        const = ctx.enter_context(tc.tile_pool(name="const", bufs=1))
        vtp = ctx.enter_context(tc.tile_pool(name="vtp", bufs=16))
        cump = ctx.enter_context(tc.tile_pool(name="cump", bufs=3))
        gp = ctx.enter_context(tc.tile_pool(name="gp", bufs=2))
        rcp = ctx.enter_context(tc.tile_pool(name="rcp", bufs=2))
        small = ctx.enter_context(tc.tile_pool(name="small", bufs=4))

        ident = const.tile([128, 128], F32)
        make_identity(nc, ident)

        # qt-chain inputs lead the sync (SP) queue in consumption order:
        # wpsi, then the four agT chunks (plain slices), then wphi; rowbase
        # rides the scalar queue. Everything is f32 here — q/t must be
        # exact, only the stationary tT and the vis stream go fp16.
        # wpsi arrives pre-chunked from the host as [p, r, k] = wpsi[r*128+p, k]
        # so every partition line is one contiguous 2KB descriptor.
        wpsi_sb = const.tile([128, NDC, K], F32)
        nc.sync.dma_start(out=wpsi_sb, in_=wpsi_d)
        wphi_sb = const.tile([128, NDC, K], F32)
        nc.sync.dma_start(out=wphi_sb, in_=wphi_d)
        agT_tiles = []
        for r in range(NDC):
            agt = const.tile([128, BE], F32, tag=f"agT{r}", name=f"agT{r}")
            nc.sync.dma_start(out=agt, in_=agT_d[r * 128:(r + 1) * 128, :])
            agT_tiles.append(agt)
        rb_sb = const.tile([128, NBC], F32)
        nc.scalar.dma_start(out=rb_sb, in_=rb_d)

        # visT16 stream on the sync queue, in consumption order (c outer,
        # d-chunk inner). bufs=16 keeps every transfer enqueued from t=0.
        vt_tiles = {}
        for c in range(NBC):
            for r in range(NDC):
                vt = vtp.tile([128, 2048], F16, tag="vt", name=f"vt{c}_{r}")
                nc.sync.dma_start(
                    out=vt, in_=vt_d[r * 128:(r + 1) * 128, c * 2048:(c + 1) * 2048])
                vt_tiles[(c, r)] = vt

        # Prologue PSUM lives in its own pools, closed before the main
        # cross-product pool opens (which needs all 8 banks).
        t_tiles = []
        tT_tiles = []
        with tc.tile_pool(name="pro_tr", bufs=2, space="PSUM") as pro_tr, \
             tc.tile_pool(name="pro_mm", bufs=2, space="PSUM") as pro_mm:
            # A couple of PE warmups; the prologue matmuls finish the HAM
            # clock ramp themselves — more warmups cost more than they save.
            for wup in range(2):
                warm = pro_tr.tile([128, 128], F32, tag="tr", name=f"warm{wup}")
                nc.tensor.transpose(warm, ident, ident)

            # w_phiT with natural dout order: wphiT[k, dl, r] = w_phi[4*dl+r, k],
            # flat free index f = dl*4 + r = dout.
            wphiT_sb = const.tile([128, 128, NDC], F32)
            for r in range(NDC):
                tr = pro_tr.tile([128, 128], F32, tag="tr", name=f"trw{r}")
                nc.tensor.transpose(tr, wphi_sb[:, r, :], ident)
                nc.scalar.copy(wphiT_sb[:, :, r], tr)

            # qT[k, be] = sum_r w_psi_chunk_r.T @ agT_chunk_r  (plain d-chunks)
            qt_ps = pro_mm.tile([128, BE], F32, tag="qt", name="qt")
            for r in range(NDC):
                nc.tensor.matmul(
                    qt_ps, lhsT=wpsi_sb[:, r, :], rhs=agT_tiles[r],
                    start=(r == 0), stop=(r == NDC - 1))
            qT_sb = const.tile([128, BE], F32)
            nc.scalar.copy(qT_sb, qt_ps)

            # tT16[r][dp, be] = fp16(t[be, r*128+dp]) — the fp16 stationary.
            for r in range(NDC):
                tt_ps = pro_mm.tile([128, BE], F32, tag="tt", name=f"tt{r}")
                wslice = wphiT_sb[:, r * 32:(r + 1) * 32, :].rearrange(
                    "p a b -> p (a b)")
                nc.tensor.matmul(tt_ps, lhsT=wslice, rhs=qT_sb, start=True, stop=True)
                tt16 = const.tile([128, BE], F16, tag=f"tt16_{r}", name=f"tt16_{r}")
                if r % 2 == 0:
                    nc.scalar.copy(tt16, tt_ps)
                else:
                    nc.vector.tensor_copy(tt16, tt_ps)
                tT_tiles.append(tt16)

        # Main PSUM: cross half-tiles (2 banks each, 3 in flight) plus a
        # 1-bank slot for the per-chunk t matmul (needed only by refine,
        # computed inside the main loop to stay off the prologue chain).
        bx_pool = ctx.enter_context(tc.tile_pool(name="bx", bufs=3, space="PSUM"))
        t_pool = ctx.enter_context(tc.tile_pool(name="tp", bufs=1, space="PSUM"))

        beta = {}
        topk = {}
        exps = {}

        def emit_mm(c):
            """Cross half-tiles bxh[be', (be, a-half)] = sum_d tT16 visT16,
            one per a-half so each PSUM tile is 2 banks. The per-chunk t
            (f32, for refine) rides along after the halves."""
            bxs = []
            for h in range(2):
                bxh = bx_pool.tile([128, 1024], F32, tag="bx", name=f"bx{c}_{h}")
                for r in range(NDC):
                    vt_v = vt_tiles[(c, r)].rearrange("p (b a) -> p b a", a=A)
                    for s in range(2):
                        nc.tensor.matmul(
                            bxh[:, s * 512:(s + 1) * 512],
                            lhsT=tT_tiles[r][:, c * 128:(c + 1) * 128],
                            rhs=vt_v[:, s * 64:(s + 1) * 64, h * 8:(h + 1) * 8],
                            start=(r == 0), stop=(r == NDC - 1))
                bxs.append(bxh)
            t_ps = t_pool.tile([128, D], F32, tag="t", name=f"t{c}")
            nc.tensor.matmul(
                t_ps, lhsT=qT_sb[:, c * 128:(c + 1) * 128],
                rhs=wphiT_sb[:, :, :], start=True, stop=True)
            t_sb = const.tile([128, D], F32, tag=f"t{c}", name=f"tsb{c}")
            nc.scalar.copy(t_sb, t_ps)
            t_tiles.append(t_sb)
            return bxs

        def emit_beta(c, bxs):
            """Masked cumsum diagonal extraction. a-major AP over each PSUM
            half-tile x identity mask: the running sum increments by
            beta[p, a] once per 128-element page, so page-boundary values
            (fixed positions) are prefix sums of the betas."""
            beta_sb = small.tile([128, A], F32, tag="beta", name=f"beta{c}")
            for h in range(2):
                cum = cump.tile([128, 8, 128], F32, tag="cum", name=f"cum{c}_{h}")
                nc.vector._custom_dve(
                    CUMSUM_MUL, out=cum,
                    in0=bxs[h].rearrange("p (b a) -> p a b", a=8),
                    in1=_bcast_mid(ident, 8), s0=0.0)
                bnd = cum[:, :, 127:128].rearrange("p s o -> p (s o)")
                o = h * 8
                nc.vector.tensor_copy(beta_sb[:, o:o + 1], bnd[:, 0:1])
                nc.vector.tensor_sub(
                    beta_sb[:, o + 1:o + 8], bnd[:, 1:8], bnd[:, 0:7])
            beta[c] = beta_sb

        def emit_topk_gather(c):
            """DVE top-2 (value, index) extraction; DVE casts the indices and
            GPSIMD runs the two indirect f32 row gathers. The first gather is
            issued as soon as i0 is known, before the second max pass."""
            scr = small.tile([128, A], F32, tag="scr", name=f"scr{c}")
            m0 = small.tile([128, 1], F32, tag="m0", name=f"m0{c}")
            m1 = small.tile([128, 1], F32, tag="m1", name=f"m1{c}")
            idxg = small.tile([128, 2], F32, tag="idxg", name=f"idxg{c}")
            idxi = small.tile([128, 2], I32, tag="idxi", name=f"idxi{c}")
            rb = rb_sb[:, c:c + 1]
            g = gp.tile([128, 2, D], F32, tag="g", name=f"g{c}")
            nc.vector._custom_dve(BMAX, out=scr, in0=beta[c], accum_out=m0)
            nc.vector._custom_dve(IDX0RB, out=scr, in0=beta[c], s0=m0, s1=rb,
                                  imm2=BIG, accum_out=idxg[:, 0:1])
            nc.scalar.copy(idxi[:, 0:1], idxg[:, 0:1])
            nc.gpsimd.indirect_dma_start(
                out=g[:, 0, :], out_offset=None, in_=vf_d,
                in_offset=bass.IndirectOffsetOnAxis(ap=idxi[:, 0:1], axis=0),
            )
            nc.vector._custom_dve(M2RB, out=scr, in0=beta[c], s0=idxg[:, 0:1],
                                  s1=rb, accum_out=m1)
            i0l = small.tile([128, 1], F32, tag="i0l", name=f"i0l{c}")
            nc.vector.tensor_sub(i0l, idxg[:, 0:1], rb)
            i1l = small.tile([128, 1], F32, tag="i1l", name=f"i1l{c}")
            nc.vector._custom_dve(IDX1, out=scr, in0=beta[c], s0=m1,
                                  s1=i0l, imm2=BIG, accum_out=i1l)
            nc.vector.tensor_add(idxg[:, 1:2], i1l, rb)
            nc.scalar.copy(idxi[:, 1:2], idxg[:, 1:2])
            nc.gpsimd.indirect_dma_start(
                out=g[:, 1, :], out_offset=None, in_=vf_d,
                in_offset=bass.IndirectOffsetOnAxis(ap=idxi[:, 1:2], axis=0),
            )
            topk[c] = (idxg, g, m0)

        def emit_refine(c):
            """Recompute the two top betas exactly from the gathered f32 rows
            (separate scans so each starts as soon as its row lands) and
            overwrite them in place."""
            idxg, g, _ = topk[c]
            rb = rb_sb[:, c:c + 1]
            rc = rcp.tile([128, 2, D], F32, tag="rc", name=f"rc{c}")
            for k in range(2):
                nc.vector._custom_dve(
                    CUMSUM_MUL, out=rc[:, k:k + 1, :], in0=g[:, k:k + 1, :],
                    in1=_bcast_mid(t_tiles[c], 1))
                nc.vector._custom_dve(PATCH_SET_RB, out=beta[c], in0=beta[c],
                                      in1=rb, s0=idxg[:, k:k + 1],
                                      s1=rc[:, k, D - 1:D])

        def emit_exp(c):
            # exp bias needs only ~max(beta); the pre-refine max m0 is within
            # ~sigma of it. Negate on the Scalar engine.
            negm = small.tile([128, 1], F32, tag="negm", name=f"negm{c}")
            nc.scalar.mul(negm, topk[c][2], -1.0)
            prob = small.tile([128, A], F32, tag="prob", name=f"prob{c}")
            ssum = small.tile([128, 1], F32, tag="ssum", name=f"ssum{c}")
            nc.scalar.activation(
                prob, beta[c], mybir.ActivationFunctionType.Exp,
                bias=negm, scale=1.0, accum_out=ssum,
            )
            exps[c] = (prob, ssum)

        def emit_finish(c):
            cs = slice(c * 128, (c + 1) * 128)
            prob, ssum = exps[c]
            rec = small.tile([128, 1], F32, tag="rec", name=f"rec{c}")
            nc.vector.reciprocal(rec, ssum)
            osb = small.tile([128, A], F32, tag="osb", name=f"osb{c}")
            nc.scalar.mul(osb, prob, rec)
            nc.scalar.dma_start(out=out_d[cs, :], in_=osb)

        # Software pipeline: chunk c's refine (which waits on its gather
        # round-trip) runs TWO chunks later, in its own tick after chunk
        # c+2's extraction, so the in-order DVE never stalls on a gather;
        # finishes trail one more chunk.
        for c in range(NBC):
            with tc.tile_wait_until(2 * c):
                bx = emit_mm(c)
                emit_beta(c, bx)
                emit_topk_gather(c)
            with tc.tile_wait_until(2 * c + 1):
                if c >= 2:
                    emit_refine(c - 2)
                    emit_exp(c - 2)
                if c >= 3:
                    emit_finish(c - 3)
        with tc.tile_wait_until(2 * NBC):
            emit_refine(NBC - 2)
            emit_exp(NBC - 2)
            emit_finish(NBC - 3)
        with tc.tile_wait_until(2 * NBC + 1):
            emit_refine(NBC - 1)
            emit_exp(NBC - 1)
            emit_finish(NBC - 2)
            emit_finish(NBC - 1)


def _build_program():
    nc = bacc.Bacc("TRN2", target_bir_lowering=False, debug=False)
    agT_d = nc.dram_tensor("agT", (D, BE), F32, kind="ExternalInput").ap()
    vt_d = nc.dram_tensor("vt", (D, BE * A), F16, kind="ExternalInput").ap()
    vf_d = nc.dram_tensor("vis_f", (BE * A, D), F32, kind="ExternalInput").ap()
    wpsi_d = nc.dram_tensor("w_psi", (128, NDC * K), F32, kind="ExternalInput").ap()
    wphi_d = nc.dram_tensor("w_phi", (128, NDC * K), F32, kind="ExternalInput").ap()
    rb_d = nc.dram_tensor("rowbase", (128, NBC), F32, kind="ExternalInput").ap()
    out_d = nc.dram_tensor("out", (BE, A), F32, kind="ExternalOutput").ap()
    with tile.TileContext(nc) as tc:
        _emit(tc, nc, agT_d, vt_d, vf_d, wpsi_d, wphi_d, rb_d, out_d)
    nc.compile()
    return nc


_PROG = None


def _get_program():
    global _PROG
    if _PROG is None:
        _PROG = _build_program()
    return _PROG


_ROWBASE = (
    (np.arange(NBC, dtype=np.float32)[None, :] * 128
     + np.arange(128, dtype=np.float32)[:, None]) * A
).astype(np.float32)


def make_in_maps(agent_observation, visible_observations, w_psi, w_phi):
    agent = np.ascontiguousarray(np.asarray(agent_observation, np.float32)).reshape(B, E, D)
    vis = np.ascontiguousarray(np.asarray(visible_observations, np.float32)).reshape(B, E, A, D)
    wpsi = np.asarray(w_psi, np.float32)
    # pre-chunked layout: wpsi_x[p, r*K+k] = wpsi[r*128+p, k] (2KB DMA lines)
    wpsi_x = np.ascontiguousarray(
        wpsi.reshape(NDC, 128, K).transpose(1, 0, 2).reshape(128, NDC * K))
    wphi = np.asarray(w_phi, np.float32)
    # interleaved layout: wphi_x[p, r*K+k] = wphi[4p+r, k] (2KB DMA lines)
    wphi_x = np.ascontiguousarray(wphi.reshape(128, NDC * K))
    in_maps = []
    for ci in range(N_CORES):
        sl = slice(ci * B_SH, (ci + 1) * B_SH)
        v = vis[sl].reshape(BE, A, D)
        v16 = v.astype(np.float16)
        vt = np.ascontiguousarray(v16.transpose(2, 0, 1).reshape(D, BE * A))
        agT = np.ascontiguousarray(agent[sl].reshape(BE, D).T)
        in_maps.append({
            "agT": agT,
            "vt": vt,
            "vis_f": np.ascontiguousarray(v.reshape(BE * A, D)),
            "w_psi": wpsi_x,
            "w_phi": wphi_x,
            "rowbase": _ROWBASE,
        })
    return in_maps


def run_sharded(in_maps, trace=False, **kwargs):
    nc = _get_program()
    return bass_utils.run_bass_kernel_spmd(
        nc, in_maps, core_ids=list(range(N_CORES)), trace=trace, **kwargs
    )


def kernel(agent_observation, visible_observations, w_psi, w_phi):
    in_maps = make_in_maps(agent_observation, visible_observations, w_psi, w_phi)
    res = run_sharded(in_maps)
    return np.concatenate(
        [r["out"].reshape(B_SH, E, A) for r in res.results], axis=0
    )
